# revision 16
# baseline (speedup 1.0000x reference)
"""Trainium2 Bass kernel for nn_Attention (B=2, S=2048, D=1024, H=16).

Sharding: tensor-parallel over heads. Each of the 8 cores owns 2 heads
(both batches): it computes q,k,v projections for its head columns, full
attention for its 4 (batch, head) pairs, and a partial output projection
(contraction over its 128 head-output columns). The host sums the 8
bf16 partials and adds b_proj.

Design (v2): the ScalarE exp is the hard floor (~16.8M score elements
per core at 1 elem/lane/cycle = ~110us + per-ACT overhead). Emission is
a flat stream with Tile-scheduler priority classes instead of generator
interleaving:
 - scores + exp ACTs: HIGH priority, so freed PSUM score buffers are
   immediately refilled and ScalarE never starves;
 - PV / normalize: normal priority;
 - stage A of the other batch and both stage-C passes: LOW priority,
   filling PE gaps (this also keeps the PE HAM clock at 2.4 GHz -- full
   128x128 matmuls are spread through the attention phase, so the
   explicit heater of v1 is gone).
Range-level dependency tracking lets attention(b0) begin right after
stage A produces tile 0's k/v/q instead of after the whole projection.

FP8_JJ selects keyblock-pairs whose PV runs as one fp8e4 DoubleRow
matmul (K=256 virtual, 0.5 cyc/row): p and v' quantized to e4m3.
exp outputs are shifted by -PSHIFT so p fits e4m3 range.
"""

import sys

sys.path.insert(0, "/opt/trn_rl_repo")

import numpy as np
import ml_dtypes

B, S, D, H, HD = 2, 2048, 1024, 16, 64
NCORES = 8
HPC = H // NCORES  # heads per core = 2
BS = B * S  # 4096
KB = S // 128  # key blocks per batch = 16
QT = 512  # query tile
NQT = S // QT  # query tiles per batch = 4
DC = D // 128  # contraction chunks = 8
PSHIFT = 3.0  # exp(s/8 + mask - PSHIFT): keeps p in fp8e4 range

# keyblock-pair indices (0..KB//2-1) whose PV matmuls run in fp8e4
# DoubleRow mode (empty tuple = all bf16)
FP8_JJ = (0, 3, 6)

HIGH = 3_000_000
LOW = -3_000_000

BF16 = ml_dtypes.bfloat16

_cache = {}


def _build(uniform_bias, fp8_jj):
    import concourse.mybir as mybir
    import concourse.tile as tile
    from concourse import bacc
    from concourse.masks import make_identity

    fp32 = mybir.dt.float32
    bf16 = mybir.dt.bfloat16
    f8e4 = mybir.dt.float8e4
    EXP = mybir.ActivationFunctionType.Exp
    DR = mybir.MatmulPerfMode.DoubleRow

    fp8_jj = frozenset(fp8_jj) if uniform_bias is not None else frozenset()

    nc = bacc.Bacc("TRN2", target_bir_lowering=False, debug=False,
                   num_devices=NCORES)

    xt_d = nc.dram_tensor("xt", [D, BS], bf16, kind="ExternalInput").ap()
    wq_d = nc.dram_tensor("wq", [D, 128], bf16, kind="ExternalInput").ap()
    wk_d = nc.dram_tensor("wk", [D, 128], bf16, kind="ExternalInput").ap()
    wv_d = nc.dram_tensor("wv", [D, 128], bf16, kind="ExternalInput").ap()
    bq_d = nc.dram_tensor("bq", [128, 1], fp32, kind="ExternalInput").ap()
    bk_d = nc.dram_tensor("bk", [128, 1], fp32, kind="ExternalInput").ap()
    bv_d = nc.dram_tensor("bv", [128, 1], fp32, kind="ExternalInput").ap()
    wp_d = nc.dram_tensor("wp", [128, D], bf16, kind="ExternalInput").ap()
    mk_d = nc.dram_tensor("maskt", [128, B * KB], fp32, kind="ExternalInput").ap()
    out_d = nc.dram_tensor("out", [BS, D], bf16, kind="ExternalOutput").ap()

    n_vp8 = 2 * HPC * len(fp8_jj)
    n_vp16 = 2 * HPC * 2 * (KB // 2 - len(fp8_jj))

    with tile.TileContext(nc) as tc:
        with (
            tc.tile_pool(name="const", bufs=1) as cpool,
            tc.tile_pool(name="xt", bufs=2 * DC) as xpool,
            tc.tile_pool(name="qkv", bufs=2) as qkvpool,
            tc.tile_pool(name="vp", bufs=max(n_vp16, 1)) as vppool,
            tc.tile_pool(name="vp8", bufs=max(n_vp8, 1)) as vp8pool,
            tc.tile_pool(name="pt", bufs=8) as ptpool,
            tc.tile_pool(name="otn", bufs=2) as otnpool,
            tc.tile_pool(name="small", bufs=4) as smpool,
            tc.tile_pool(name="vst", bufs=4) as vstpool,
            tc.tile_pool(name="cout", bufs=6) as coutpool,
            tc.tile_pool(name="ps_a", bufs=2, space="PSUM") as ps_a,
            tc.tile_pool(name="ps_st", bufs=2, space="PSUM") as ps_st,
            tc.tile_pool(name="ps_ot", bufs=2, space="PSUM") as ps_ot,
        ):
            # ---- constants (k weights first: stage A runs k,v,q) ----
            wk_sb = cpool.tile([128, DC, 128], bf16)
            wv_sb = cpool.tile([128, DC, 128], bf16)
            wq_sb = cpool.tile([128, DC, 128], bf16)
            bk_sb = cpool.tile([128, 1], fp32)
            bv_sb = cpool.tile([128, 1], fp32)
            bq_sb = cpool.tile([128, 1], fp32)
            for w_sb, w_d in ((wk_sb, wk_d), (wv_sb, wv_d), (wq_sb, wq_d)):
                nc.gpsimd.dma_start(w_sb[:], w_d.rearrange("(c p) m -> p c m", p=128))
            for b_sb, b_d in ((bk_sb, bk_d), (bv_sb, bv_d), (bq_sb, bq_d)):
                nc.gpsimd.dma_start(b_sb[:], b_d)
            ident = cpool.tile([128, 128], bf16)
            make_identity(nc, ident[:])
            ubias_sb = cpool.tile([128, 1], fp32)
            if uniform_bias is not None:
                nc.gpsimd.memset(ubias_sb[:], uniform_bias)
            mk_sb = cpool.tile([128, B * KB], fp32)
            nc.gpsimd.dma_start(mk_sb[:], mk_d)
            if uniform_bias is None:
                # non-uniform mask: shift by -PSHIFT on device (softmax-
                # invariant)
                nc.vector.tensor_scalar_add(mk_sb[:], mk_sb[:], -PSHIFT)
            wp_sb = cpool.tile([128, D], bf16)
            nc.gpsimd.dma_start(wp_sb[:], wp_d)

            qkvs = {}
            vps = {}
            otns = {}

            def emit_vp_pair(b, h, jj):
                """v' tiles for keyblock pair jj of head h (PE
                transpose): either one fp8 DoubleRow-packed [128, 2x80]
                tile or two bf16 [128, 65] tiles. Row 64 of each 65-wide
                group is ones (softmax denominator accumulator)."""
                vT = qkvs[b][2]
                hs = slice(h * 64, (h + 1) * 64)
                idh = ident[hs, hs]
                if jj in fp8_jj:
                    vp = vp8pool.tile([128, 2 * 80], f8e4, tag="vp8",
                                      name=f"vp8_{b}_{h}_{jj}")
                    for ji, jx in ((0, 2 * jj), (1, 2 * jj + 1)):
                        vtr = ps_a.tile([128, 64], bf16, tag="a", name="vtr")
                        nc.tensor.transpose(
                            vtr[:], vT[hs, jx * 128:(jx + 1) * 128], idh)
                        nc.vector.tensor_copy(
                            vp[:, ji * 80:ji * 80 + 64], vtr[:])
                        nc.gpsimd.memset(
                            vp[:, ji * 80 + 64:ji * 80 + 65], 1.0)
                    vps[(b, h, jj)] = vp
                else:
                    pair = []
                    for jx in (2 * jj, 2 * jj + 1):
                        vtr = ps_a.tile([128, 64], bf16, tag="a", name="vtr")
                        nc.tensor.transpose(
                            vtr[:], vT[hs, jx * 128:(jx + 1) * 128], idh)
                        vp = vppool.tile([128, 65], bf16, tag="vp",
                                         name=f"vp_{b}_{h}_{jx}")
                        nc.vector.tensor_copy(vp[:, 0:64], vtr[:])
                        nc.gpsimd.memset(vp[:, 64:65], 1.0)
                        pair.append(vp)
                    vps[(b, h, jj)] = tuple(pair)

            def emit_a(b):
                """Stage A for batch b: qT/kT/vT [128, S] (2 heads
                stacked) + v' tiles, per query tile in k,v,q order so
                attention can begin as soon as tile 0 is projected."""
                qT = qkvpool.tile([128, S], bf16, tag="qT", name=f"qT_{b}")
                kT = qkvpool.tile([128, S], bf16, tag="kT", name=f"kT_{b}")
                vT = qkvpool.tile([128, S], bf16, tag="vT", name=f"vT_{b}")
                qkvs[b] = (qT, kT, vT)
                for t in range(NQT):
                    ts = slice(t * QT, (t + 1) * QT)
                    xts = []
                    for c in range(DC):
                        xt = xpool.tile([128, QT], bf16, tag="xt", name="xt")
                        nc.sync.dma_start(
                            xt[:], xt_d[c * 128:(c + 1) * 128,
                                        b * S + t * QT: b * S + (t + 1) * QT])
                        xts.append(xt)
                    for (dst, w_sb, b_sb) in (
                        (kT, wk_sb, bk_sb),
                        (vT, wv_sb, bv_sb),
                        (qT, wq_sb, bq_sb),
                    ):
                        a_ps = ps_a.tile([128, QT], fp32, tag="a", name="a_ps")
                        for c in range(DC):
                            nc.tensor.matmul(a_ps[:], w_sb[:, c, :], xts[c][:],
                                             start=(c == 0), stop=(c == DC - 1))
                        nc.vector.tensor_scalar_add(dst[:, ts], a_ps[:], b_sb[:])
                    for h in range(HPC):
                        for jj in (2 * t, 2 * t + 1):
                            emit_vp_pair(b, h, jj)

            def emit_attention(b):
                qT, kT, vT = qkvs[b]
                otn = otnpool.tile([128, S], bf16, tag="otn", name=f"otn_{b}")
                otns[b] = otn
                for t in range(NQT):
                    tq = slice(t * QT, (t + 1) * QT)
                    ot_ps = [ps_ot.tile([65, QT], fp32, tag="ot",
                                        name=f"ot_{b}_{t}_{h}")
                             for h in range(HPC)]
                    for jj in range(KB // 2):
                        j0, j1 = 2 * jj, 2 * jj + 1
                        with tc.high_priority(offset=HIGH):
                            # pt layout: [j0h0 | j0h1 | j1h0 | j1h1]
                            ptdt = f8e4 if jj in fp8_jj else bf16
                            pt = ptpool.tile([128, 4 * QT], ptdt,
                                             tag="pt", name="pt")
                            for ji, jx in ((0, j0), (1, j1)):
                                # one st tile per keyblock holds BOTH
                                # heads: their K=64 score matmuls become
                                # ready together (one ACT frees the
                                # slot) and run concurrently in disjoint
                                # PE row-groups (rows 0-63 / 64-127).
                                st = ps_st.tile([128, 2 * QT], fp32,
                                                tag="st", name="st")
                                for h in range(HPC):
                                    hs = slice(h * 64, (h + 1) * 64)
                                    nc.tensor.matmul(
                                        st[:, h * QT:(h + 1) * QT],
                                        kT[hs, jx * 128:(jx + 1) * 128],
                                        qT[hs, tq], start=True, stop=True)
                                bias = (ubias_sb[:] if uniform_bias is not None
                                        else mk_sb[:, b * KB + jx:
                                                   b * KB + jx + 1])
                                nc.scalar.activation(
                                    pt[:, 2 * ji * QT:2 * (ji + 1) * QT],
                                    st[:], EXP, bias=bias, scale=0.125)
                        first = (jj == 0)
                        last = (jj == KB // 2 - 1)
                        for h in range(HPC):
                            if jj in fp8_jj:
                                w = (vps[(b, h, jj)][:]
                                     .rearrange("p (k m) -> p k m", k=2)
                                     [:, :, 0:65])
                                r = (pt[:].rearrange("p (k x) -> p k x", k=2)
                                     [:, :, h * QT:(h + 1) * QT])
                                nc.tensor.matmul(ot_ps[h][:], w, r,
                                                 start=first, stop=last,
                                                 perf_mode=DR)
                            else:
                                vpa, vpb = vps[(b, h, jj)]
                                nc.tensor.matmul(
                                    ot_ps[h][:], vpa[:],
                                    pt[:, h * QT:(h + 1) * QT],
                                    start=first, stop=False)
                                nc.tensor.matmul(
                                    ot_ps[h][:], vpb[:],
                                    pt[:, (2 + h) * QT:(3 + h) * QT],
                                    start=False, stop=last)
                    for h in range(HPC):
                        ll = smpool.tile([1, QT], fp32, tag="ll")
                        nc.vector.tensor_copy(ll[:], ot_ps[h][64:65, :])
                        rc = smpool.tile([1, QT], fp32, tag="rc")
                        nc.vector.reciprocal_approx_fast(rc[:], ll[:])
                        bc = smpool.tile([64, QT], fp32, tag="bc")
                        nc.gpsimd.partition_broadcast(bc[:], rc[:])
                        if h == 0:
                            nc.vector.tensor_mul(otn[0:64, tq],
                                                 ot_ps[h][0:64, :], bc[:])
                        else:
                            hi = smpool.tile([64, QT], bf16, tag="hi")
                            nc.vector.tensor_mul(hi[:], ot_ps[h][0:64, :],
                                                 bc[:])
                            nc.gpsimd.dma_start(otn[64:128, tq], hi[:])

            def emit_c(b):
                """Partial out-projection for batch b -> bf16 partials.
                One full-width [128, D] store per rowblock (contiguous
                2KB DRAM rows beat two half-width stores)."""
                otn = otns[b]
                for r in range(S // 128):
                    co = coutpool.tile([128, D], bf16, tag="co")
                    for n in range(D // QT):
                        c_ps = ps_a.tile([128, QT], fp32, tag="a", name="c_ps")
                        nc.tensor.matmul(c_ps[:],
                                         otn[:, r * 128:(r + 1) * 128],
                                         wp_sb[:, n * QT:(n + 1) * QT],
                                         start=True, stop=True)
                        # late b1 rows drain after the exp stream ends:
                        # split their PSUM->SBUF copies across ScalarE
                        # (idle by then) and VectorE to halve the drain
                        if b == 1 and r >= 8 and n == 0:
                            nc.scalar.activation(
                                co[:, n * QT:(n + 1) * QT], c_ps[:],
                                mybir.ActivationFunctionType.Copy,
                                bias=0.0, scale=1.0)
                        else:
                            nc.vector.tensor_copy(
                                co[:, n * QT:(n + 1) * QT], c_ps[:])
                    nc.sync.dma_start(
                        out_d[b * S + r * 128: b * S + (r + 1) * 128, :],
                        co[:])

            emit_a(0)
            with tc.high_priority(offset=LOW):
                emit_a(1)
            emit_attention(0)
            with tc.high_priority(offset=LOW):
                emit_c(0)
            emit_attention(1)
            with tc.high_priority(offset=LOW):
                emit_c(1)

    nc.compile()
    return nc


def _prep_inputs(x, attention_mask, w_attn, b_attn, w_proj):
    xT = np.ascontiguousarray(
        np.asarray(x, dtype=np.float32).reshape(BS, D).T).astype(BF16)
    maskt = np.ascontiguousarray(
        np.asarray(attention_mask, dtype=np.float32)
        .reshape(B, KB, 128).transpose(2, 0, 1).reshape(128, B * KB))
    w_attn = np.asarray(w_attn, dtype=np.float32)
    b_attn = np.asarray(b_attn, dtype=np.float32)
    w_proj = np.asarray(w_proj, dtype=np.float32)
    in_maps = []
    for c in range(NCORES):
        lo, hi = 2 * c * HD, (2 * c + 2) * HD
        in_maps.append({
            "xt": xT,
            "wq": np.ascontiguousarray(w_attn[:, lo:hi]).astype(BF16),
            "wk": np.ascontiguousarray(w_attn[:, D + lo: D + hi]).astype(BF16),
            "wv": np.ascontiguousarray(w_attn[:, 2 * D + lo: 2 * D + hi]).astype(BF16),
            "bq": np.ascontiguousarray(b_attn[lo:hi].reshape(128, 1)),
            "bk": np.ascontiguousarray(b_attn[D + lo: D + hi].reshape(128, 1)),
            "bv": np.ascontiguousarray(b_attn[2 * D + lo: 2 * D + hi].reshape(128, 1)),
            "wp": np.ascontiguousarray(w_proj[lo:hi, :]).astype(BF16),
            "maskt": maskt,
        })
    return in_maps


def _run(in_maps, trace=False, tmpdir=None):
    from concourse import bass_utils
    mk = in_maps[0]["maskt"]
    uniform = float(mk.flat[0]) if np.all(mk == mk.flat[0]) else None
    ub = None if uniform is None else uniform - PSHIFT
    key = ("nc", ub, FP8_JJ)
    if key not in _cache:
        _cache[key] = _build(ub, FP8_JJ)
    return bass_utils.run_bass_kernel_spmd(
        _cache[key], in_maps, core_ids=list(range(NCORES)),
        trace=trace, tmpdir=tmpdir)


def kernel(x, attention_mask, w_attn, b_attn, w_proj, b_proj):
    in_maps = _prep_inputs(x, attention_mask, w_attn, b_attn, w_proj)
    res = _run(in_maps)
    out = np.zeros((BS, D), dtype=np.float32)
    for c in range(NCORES):
        out += res.results[c]["out"].astype(np.float32)
    out += np.asarray(b_proj, dtype=np.float32)[None, :]
    return out.reshape(B, S, D)
